# revision 1
# baseline (speedup 1.0000x reference)
"""GRU cell kernel for Trainium2, data-parallel over 8 NeuronCores.

Math (per reference):
    z = sigmoid(x @ wz.T + h @ uz.T + bz)
    r = sigmoid(x @ wr.T + h @ ur.T + br)
    g = tanh(x @ wh.T + (r*h) @ uh.T + bh)
    out = (1-z)*h + z*g = h + z*(g - h)

Everything on-device is computed in TRANSPOSED layout ([feature, row]),
so that both matmul operands arrive with the contraction dim on
partitions without any on-device transpose:
    outT = f(W_T_block.T @ xT)  with W_T = W.T ([in, out]) prepped on host.
The host transposes x/h/W on the way in and the output on the way out.

Sharding: rows 16384 -> 8 cores x 2048 rows, weights replicated.
"""

import numpy as np
import ml_dtypes
from contextlib import ExitStack

import concourse.bass as bass
import concourse.bacc as bacc
import concourse.mybir as mybir
import concourse.tile as tile
from concourse.bass_utils import run_bass_kernel_spmd

H = 1024
N_ROWS = 16384
NCORES = 8
P = 128
KB = H // P            # 8 contraction blocks
MB = H // P            # 8 output-feature blocks
NS = 512               # rows per matmul moving slice (one PSUM bank)

BF = mybir.dt.bfloat16
F32 = mybir.dt.float32
AF = mybir.ActivationFunctionType
bf16 = ml_dtypes.bfloat16

# Set by test harness to capture a trace; harness-facing default off.
TRACE = False
LAST_RESULT = None


def build_nc(R=N_ROWS // NCORES, CH=2):
    """Build the per-core Bass program. R rows per core, CH row-chunks."""
    RC = R // CH           # rows per chunk
    SL = RC // NS          # moving slices per chunk

    nc = bacc.Bacc(trn_type="TRN2", target_bir_lowering=False,
                   debug=False, enable_asserts=False)

    xT = nc.dram_tensor("xT", [H, R], BF, kind="ExternalInput").ap()
    hTb = nc.dram_tensor("hTb", [H, R], BF, kind="ExternalInput").ap()
    hTf = nc.dram_tensor("hTf", [H, R], F32, kind="ExternalInput").ap()
    wd = {
        nm: nc.dram_tensor(nm, [H, H], BF, kind="ExternalInput").ap()
        for nm in ("wzT", "uzT", "wrT", "urT", "whT", "uhT")
    }
    bias = nc.dram_tensor("bias", [P, 3 * MB], F32, kind="ExternalInput").ap()
    outT = nc.dram_tensor("outT", [H, R], F32, kind="ExternalOutput").ap()

    with tile.TileContext(nc) as tc, ExitStack() as ctx:
        wpool = ctx.enter_context(tc.tile_pool(name="w", bufs=32))
        xpool = ctx.enter_context(tc.tile_pool(name="x", bufs=2))
        hbpool = ctx.enter_context(tc.tile_pool(name="hb", bufs=1))
        hfpool = ctx.enter_context(tc.tile_pool(name="hf", bufs=2))
        rhpool = ctx.enter_context(tc.tile_pool(name="rh", bufs=MB + 2))
        rpool = ctx.enter_context(tc.tile_pool(name="r", bufs=6))
        zpool = ctx.enter_context(tc.tile_pool(name="z", bufs=2 * MB + 2))
        gpool = ctx.enter_context(tc.tile_pool(name="g", bufs=6))
        opool = ctx.enter_context(tc.tile_pool(name="o", bufs=2))
        cpool = ctx.enter_context(tc.tile_pool(name="c", bufs=1))
        pspool = ctx.enter_context(tc.tile_pool(name="ps", bufs=8, space="PSUM"))

        # Warm up the ACT table set (sigmoid_and_others covers tanh too) on an
        # instruction with minimal sync waits — walrus can't attach the
        # PSEUDO_LOAD_ACT_FUNC_SET to an activation that already carries two
        # sem waits ("Too many sync wait commands").
        warm = cpool.tile([P, 8], F32, tag="warm")
        nc.gpsimd.memset(warm[:], 0.0)
        nc.scalar.activation(warm[:], warm[:], AF.Sigmoid)

        bt = cpool.tile([P, 3 * MB], F32, tag="bias")
        nc.sync.dma_start(bt[:], bias[:])
        # bias column layout: [z:0..7 | r:8..15 | h:16..23]
        GZ, GR, GH = 0, 1, 2

        def load_w(name, c):
            """8 k-tiles [P, H] of one weight matrix."""
            ts = []
            for k in range(KB):
                t = wpool.tile([P, H], BF, tag="w")
                nc.sync.dma_start(t[:], wd[name][k * P:(k + 1) * P, :])
                ts.append(t)
            return ts

        def mm_group(psums, wt, ut, mov_w, mov_u, m, c):
            """Accumulate  wt.T @ mov_w + ut.T @ mov_u  for feature block m
            into psums[s] ([P, NS] each), contracting over all KB blocks."""
            msl = slice(m * P, (m + 1) * P)
            for k in range(KB):
                for s in range(SL):
                    nc.tensor.matmul(
                        psums[s][:],
                        wt[k][:, msl],
                        mov_w[:, k * RC + s * NS: k * RC + (s + 1) * NS],
                        start=(k == 0), stop=False,
                    )
            for k in range(KB):
                for s in range(SL):
                    nc.tensor.matmul(
                        psums[s][:],
                        ut[k][:, msl],
                        mov_u[:, k * RC + s * NS: k * RC + (s + 1) * NS],
                        start=False, stop=(k == KB - 1),
                    )

        for c in range(CH):
            rows = slice(c * RC, (c + 1) * RC)

            # DMA emission matches the r-pass m=0 matmul consumption order
            # (wr[k] with x[k] pairs, then ur[k] with hb[k]) so the PE can
            # start as soon as the first pair lands instead of waiting for
            # the whole 8MB initial burst to drain round-robin.
            xt = xpool.tile([P, KB * RC], BF, tag="x")
            hbt = hbpool.tile([P, KB * RC], BF, tag="hb")
            wr, ur = [], []
            for k in range(KB):
                ksl = slice(k * P, (k + 1) * P)
                t = wpool.tile([P, H], BF, tag="w", name="t")
                nc.sync.dma_start(t[:], wd["wrT"][ksl, :])
                wr.append(t)
                nc.sync.dma_start(xt[:, k * RC:(k + 1) * RC], xT[ksl, rows])
            for k in range(KB):
                ksl = slice(k * P, (k + 1) * P)
                t = wpool.tile([P, H], BF, tag="w", name="t")
                nc.sync.dma_start(t[:], wd["urT"][ksl, :])
                ur.append(t)
                nc.sync.dma_start(hbt[:, k * RC:(k + 1) * RC], hTb[ksl, rows])

            # ---- r pass ----
            rhs = []
            for m in range(MB):
                ps = [pspool.tile([P, NS], F32, tag="ps", name="ps") for _ in range(SL)]
                mm_group(ps, wr, ur, xt, hbt, m, c)
                rh = rhpool.tile([P, RC], BF, tag="rh")
                for s in range(SL):
                    rt = rpool.tile([P, NS], BF, tag="r")
                    nc.scalar.activation(rt[:], ps[s][:], AF.Sigmoid,
                                         bias=bt[:, GR * MB + m: GR * MB + m + 1])
                    nc.vector.tensor_mul(
                        rh[:, s * NS:(s + 1) * NS], rt[:],
                        hbt[:, m * RC + s * NS: m * RC + (s + 1) * NS])
                rhs.append(rh)

            # ---- z pass ----
            wz = load_w("wzT", c)
            uz = load_w("uzT", c)
            zts = []
            for m in range(MB):
                ps = [pspool.tile([P, NS], F32, tag="ps", name="ps") for _ in range(SL)]
                mm_group(ps, wz, uz, xt, hbt, m, c)
                zm = []
                for s in range(SL):
                    zt = zpool.tile([P, NS], BF, tag="z")
                    nc.scalar.activation(zt[:], ps[s][:], AF.Sigmoid,
                                         bias=bt[:, GZ * MB + m: GZ * MB + m + 1])
                    zm.append(zt)
                zts.append(zm)

            # ---- h~ pass + combine ----
            wh = load_w("whT", c)
            uh = load_w("uhT", c)
            for m in range(MB):
                msl = slice(m * P, (m + 1) * P)
                hf = hfpool.tile([P, RC], F32, tag="hf")
                nc.sync.dma_start(hf[:], hTf[msl, rows])
                ps = [pspool.tile([P, NS], F32, tag="ps", name="ps") for _ in range(SL)]
                for k in range(KB):
                    for s in range(SL):
                        nc.tensor.matmul(
                            ps[s][:], wh[k][:, msl],
                            xt[:, k * RC + s * NS: k * RC + (s + 1) * NS],
                            start=(k == 0), stop=False)
                for k in range(KB):
                    for s in range(SL):
                        nc.tensor.matmul(
                            ps[s][:], uh[k][:, msl],
                            rhs[k][:, s * NS:(s + 1) * NS],
                            start=False, stop=(k == KB - 1))
                ot = opool.tile([P, RC], F32, tag="o")
                for s in range(SL):
                    ssl = slice(s * NS, (s + 1) * NS)
                    gt = gpool.tile([P, NS], F32, tag="g")
                    nc.scalar.activation(gt[:], ps[s][:], AF.Tanh,
                                         bias=bt[:, GH * MB + m: GH * MB + m + 1])
                    # g-h ; z*(g-h) ; h + z*(g-h)
                    nc.vector.tensor_sub(gt[:], gt[:], hf[:, ssl])
                    nc.vector.tensor_mul(gt[:], zts[m][s][:], gt[:])
                    nc.vector.tensor_add(ot[:, ssl], gt[:], hf[:, ssl])
                    # per-slice store so the tail DMA streams out as each
                    # slice's combine finishes instead of all at once
                    nc.sync.dma_start(
                        outT[msl, c * RC + s * NS: c * RC + (s + 1) * NS],
                        ot[:, ssl])

    nc.compile()
    return nc


_NC_CACHE = {}


def _get_nc(R, CH):
    key = (R, CH)
    if key not in _NC_CACHE:
        _NC_CACHE[key] = build_nc(R, CH)
    return _NC_CACHE[key]


def make_in_maps(update, hidden, wz, uz, bz, wr, ur, br, wh, uh, bh,
                 ncores=NCORES):
    wmap = {
        "wzT": np.ascontiguousarray(wz.T).astype(bf16),
        "uzT": np.ascontiguousarray(uz.T).astype(bf16),
        "wrT": np.ascontiguousarray(wr.T).astype(bf16),
        "urT": np.ascontiguousarray(ur.T).astype(bf16),
        "whT": np.ascontiguousarray(wh.T).astype(bf16),
        "uhT": np.ascontiguousarray(uh.T).astype(bf16),
    }
    bias = np.empty((P, 3 * MB), np.float32)
    for g, b in enumerate((bz, br, bh)):
        bias[:, g * MB:(g + 1) * MB] = np.asarray(b, np.float32).reshape(MB, P).T
    rows = update.shape[0]
    rc = rows // ncores
    in_maps = []
    for i in range(ncores):
        sl = slice(i * rc, (i + 1) * rc)
        xTs = np.ascontiguousarray(np.asarray(update[sl], np.float32).T)
        hTs = np.ascontiguousarray(np.asarray(hidden[sl], np.float32).T)
        in_maps.append(dict(xT=xTs.astype(bf16), hTb=hTs.astype(bf16),
                            hTf=hTs, bias=bias, **wmap))
    return in_maps


def kernel(update, hidden, wz, uz, bz, wr, ur, br, wh, uh, bh):
    global LAST_RESULT
    update = np.asarray(update)
    hidden = np.asarray(hidden)
    R = update.shape[0] // NCORES
    nc = _get_nc(R, 2)
    in_maps = make_in_maps(update, hidden, wz, uz, bz, wr, ur, br, wh, uh, bh)
    res = run_bass_kernel_spmd(nc, in_maps, list(range(NCORES)), trace=TRACE)
    LAST_RESULT = res
    out = np.empty((update.shape[0], H), np.float32)
    for i in range(NCORES):
        out[i * R:(i + 1) * R] = res.results[i]["outT"].T
    return out



# revision 4
# speedup vs baseline: 1.2123x; 1.2123x over previous
"""GRU cell kernel for Trainium2, data-parallel over 8 NeuronCores.

Math (per reference):
    z = sigmoid(x @ wz.T + h @ uz.T + bz)
    r = sigmoid(x @ wr.T + h @ ur.T + br)
    g = tanh(x @ wh.T + (r*h) @ uh.T + bh)
    out = (1-z)*h + z*g = h + z*(g - h)

Everything on-device is computed in TRANSPOSED layout ([feature, row]) so
both matmul operands arrive with the contraction dim on partitions.

Mixed precision: part of the contraction runs as fp8(e4m3) DoubleRow
matmuls (2 MACs/cell/cycle, K=256 per pass), the rest as bf16. Which
k-quarters of each weight matrix are fp8 was chosen by host simulation
to keep max rel err ~0.014 (budget 2e-2):
    wr, ur, uh: all 4 quarters fp8 (r-gate error is attenuated by
        sigmoid slope and the uh moving operand r*h is small in
        magnitude)
    wz, uz, wh: quarter 0 fp8, quarters 1..3 bf16 (z-gate errors are
        amplified by (g - h), tanh has slope 1)
All weights (fp8 and bf16) are pre-scaled by 32 on host (exact in both
formats) so fp8 and bf16 products can share one PSUM accumulation; the
activation undoes it with scale=1/32.

Sharding: rows 16384 -> 8 cores x 2048 rows, weights replicated.
"""

import numpy as np
import ml_dtypes
from contextlib import ExitStack

import concourse.bass as bass
import concourse.bacc as bacc
import concourse.mybir as mybir
import concourse.tile as tile
from concourse.bass_utils import run_bass_kernel_spmd

H = 1024
N_ROWS = 16384
NCORES = 8
P = 128
KB = H // P            # 8 contraction blocks of 128
MB = H // P            # 8 output-feature blocks
NQ = 4                 # k-quarters (256 each)
NS = 512               # rows per matmul moving slice (one PSUM bank)
WSCALE = 32.0          # weight pre-scale (exact power of 2)

# fp8 k-quarters per weight matrix (first nq of 4 quarters are fp8)
NQ8 = {"wz": 1, "uz": 1, "wr": 4, "ur": 4, "wh": 1, "uh": 4}

BF = mybir.dt.bfloat16
F8 = mybir.dt.float8e4
F32 = mybir.dt.float32
AF = mybir.ActivationFunctionType
DR = mybir.MatmulPerfMode.DoubleRow
bf16 = ml_dtypes.bfloat16
f8 = ml_dtypes.float8_e4m3

# Set by test harness to capture a trace; harness-facing default off.
TRACE = False
LAST_RESULT = None


def build_nc(R=N_ROWS // NCORES):
    """Build the per-core Bass program. R rows per core, single chunk."""
    SL = R // NS           # moving slices (4 for R=2048)

    nc = bacc.Bacc(trn_type="TRN2", target_bir_lowering=False,
                   debug=False, enable_asserts=False)

    x8d = nc.dram_tensor("x8", [H, R], F8, kind="ExternalInput").ap()
    xbd = nc.dram_tensor("xb", [6 * P, R], BF, kind="ExternalInput").ap()
    h8d = nc.dram_tensor("h8", [H, R], F8, kind="ExternalInput").ap()
    hbd = nc.dram_tensor("hb", [6 * P, R], BF, kind="ExternalInput").ap()
    hfd = nc.dram_tensor("hf", [H, R], F32, kind="ExternalInput").ap()
    w8d = {}
    wbd = {}
    for nm, nq in NQ8.items():
        w8d[nm] = nc.dram_tensor(nm + "8", [nq * 2 * P, H], F8,
                                 kind="ExternalInput").ap()
        if nq < NQ:
            wbd[nm] = nc.dram_tensor(nm + "b", [(NQ - nq) * 2 * P, H], BF,
                                     kind="ExternalInput").ap()
    bias = nc.dram_tensor("bias", [P, 3 * MB], F32, kind="ExternalInput").ap()
    outT = nc.dram_tensor("outT", [H, R], F32, kind="ExternalOutput").ap()

    with tile.TileContext(nc) as tc, ExitStack() as ctx:
        wpool = ctx.enter_context(tc.tile_pool(name="w", bufs=3))
        dpool = ctx.enter_context(tc.tile_pool(name="d", bufs=1))
        hfpool = ctx.enter_context(tc.tile_pool(name="hf", bufs=2))
        rpool = ctx.enter_context(tc.tile_pool(name="r", bufs=4))
        gpool = ctx.enter_context(tc.tile_pool(name="g", bufs=2 * SL + 2))
        opool = ctx.enter_context(tc.tile_pool(name="o", bufs=4))
        cpool = ctx.enter_context(tc.tile_pool(name="c", bufs=1))
        pspool = ctx.enter_context(tc.tile_pool(name="ps", bufs=8, space="PSUM"))

        # Warm up the ACT table set (sigmoid_and_others covers tanh too) on an
        # instruction with minimal sync waits — walrus can't attach the
        # PSEUDO_LOAD_ACT_FUNC_SET to an activation that already carries two
        # sem waits ("Too many sync wait commands").
        warm = cpool.tile([P, 8], F32, tag="warm")
        nc.gpsimd.memset(warm[:], 0.0)
        nc.scalar.activation(warm[:], warm[:], AF.Sigmoid)

        bt = cpool.tile([P, 3 * MB], F32, tag="bias")
        nc.sync.dma_start(bt[:], bias[:])
        # bias column layout: [z:0..7 | r:8..15 | h:16..23]
        GZ, GR, GH = 0, 1, 2
        ISC = 1.0 / WSCALE

        # ---- SBUF data tiles + DMA in consumption order ----
        # r-pass m=0 consumes wr/x first, then ur/h; interleave so the PE
        # can start as soon as the first (weight, data) pair lands.
        xt8 = dpool.tile([P, KB, R], F8, tag="x8")
        ht8 = dpool.tile([P, KB, R], F8, tag="h8")
        xtb = dpool.tile([P, 6, R], BF, tag="xb")
        htb = dpool.tile([P, 6, R], BF, tag="hb")
        rht = dpool.tile([P, KB, R], F8, tag="rh")

        w8t = {}
        wbt = {}
        w8t["wr"] = wpool.tile([P, KB, H], F8, tag="w8", name="wr8")
        for k in range(KB):
            ksl = slice(k * P, (k + 1) * P)
            nc.sync.dma_start(w8t["wr"][:, k, :], w8d["wr"][ksl, :])
            nc.sync.dma_start(xt8[:, k, :], x8d[ksl, :])
        w8t["ur"] = wpool.tile([P, KB, H], F8, tag="w8", name="ur8")
        for k in range(KB):
            ksl = slice(k * P, (k + 1) * P)
            nc.sync.dma_start(w8t["ur"][:, k, :], w8d["ur"][ksl, :])
            nc.sync.dma_start(ht8[:, k, :], h8d[ksl, :])

        # hz-pass weights + bf16 moving data stream in during the r-pass.
        w8t["uh"] = wpool.tile([P, KB, H], F8, tag="w8", name="uh8")
        for k in range(KB):
            nc.sync.dma_start(w8t["uh"][:, k, :], w8d["uh"][k * P:(k + 1) * P, :])
        for nm in ("wh", "wz", "uz"):
            nq = NQ8[nm]
            w8t[nm] = wpool.tile([P, 2 * nq, H], F8, tag="w8q", name=nm + "8")
            for k in range(2 * nq):
                nc.sync.dma_start(w8t[nm][:, k, :], w8d[nm][k * P:(k + 1) * P, :])
            wbt[nm] = wpool.tile([P, 2 * (NQ - nq), H], BF, tag="wbq", name=nm + "b")
            for k in range(2 * (NQ - nq)):
                nc.sync.dma_start(wbt[nm][:, k, :], wbd[nm][k * P:(k + 1) * P, :])
        for k in range(6):
            nc.sync.dma_start(xtb[:, k, :], xbd[k * P:(k + 1) * P, :])
            nc.sync.dma_start(htb[:, k, :], hbd[k * P:(k + 1) * P, :])

        def mm_fp8(psums, wt, mov, m, nq, start, stop):
            """DoubleRow-accumulate wt.T @ mov for feature block m over
            fp8 k-quarters 0..nq-1."""
            msl = slice(m * P, (m + 1) * P)
            for kq in range(nq):
                for s in range(SL):
                    nc.tensor.matmul(
                        psums[s][:],
                        wt[:, 2 * kq:2 * kq + 2, msl],
                        mov[:, 2 * kq:2 * kq + 2, s * NS:(s + 1) * NS],
                        start=start and kq == 0,
                        stop=stop and kq == nq - 1,
                        perf_mode=DR,
                    )

        def mm_bf16(psums, wt, mov, m, nk, start, stop):
            """bf16-accumulate over nk k-blocks of 128."""
            msl = slice(m * P, (m + 1) * P)
            for k in range(nk):
                for s in range(SL):
                    nc.tensor.matmul(
                        psums[s][:],
                        wt[:, k, msl],
                        mov[:, k, s * NS:(s + 1) * NS],
                        start=start and k == 0,
                        stop=stop and k == nk - 1,
                    )

        # ---- r pass ----
        for m in range(MB):
            hft = hfpool.tile([P, R], F32, tag="hf")
            nc.sync.dma_start(hft[:], hfd[m * P:(m + 1) * P, :])
            ps = [pspool.tile([P, NS], F32, tag="ps", name="ps") for _ in range(SL)]
            mm_fp8(ps, w8t["wr"], xt8, m, NQ, start=True, stop=False)
            mm_fp8(ps, w8t["ur"], ht8, m, NQ, start=False, stop=True)
            for s in range(SL):
                rt = rpool.tile([P, NS], BF, tag="r")
                nc.scalar.activation(rt[:], ps[s][:], AF.Sigmoid,
                                     bias=bt[:, GR * MB + m: GR * MB + m + 1],
                                     scale=ISC)
                nc.vector.tensor_mul(
                    rht[:, m, s * NS:(s + 1) * NS], rt[:],
                    hft[:, s * NS:(s + 1) * NS])

        # ---- fused h~ / z pass + combine ----
        for m in range(MB):
            msl = slice(m * P, (m + 1) * P)
            hft = hfpool.tile([P, R], F32, tag="hf")
            nc.sync.dma_start(hft[:], hfd[msl, :])

            psA = [pspool.tile([P, NS], F32, tag="ps", name="psA") for _ in range(SL)]
            mm_fp8(psA, w8t["wh"], xt8, m, NQ8["wh"], start=True, stop=False)
            mm_bf16(psA, wbt["wh"], xtb, m, 2 * (NQ - NQ8["wh"]),
                    start=False, stop=False)
            mm_fp8(psA, w8t["uh"], rht, m, NQ, start=False, stop=True)
            gts = []
            for s in range(SL):
                gt = gpool.tile([P, NS], BF, tag="g")
                nc.scalar.activation(gt[:], psA[s][:], AF.Tanh,
                                     bias=bt[:, GH * MB + m: GH * MB + m + 1],
                                     scale=ISC)
                gts.append(gt)

            psB = [pspool.tile([P, NS], F32, tag="ps", name="psB") for _ in range(SL)]
            mm_fp8(psB, w8t["wz"], xt8, m, NQ8["wz"], start=True, stop=False)
            mm_bf16(psB, wbt["wz"], xtb, m, 2 * (NQ - NQ8["wz"]),
                    start=False, stop=False)
            mm_fp8(psB, w8t["uz"], ht8, m, NQ8["uz"], start=False, stop=False)
            mm_bf16(psB, wbt["uz"], htb, m, 2 * (NQ - NQ8["uz"]),
                    start=False, stop=True)
            for s in range(SL):
                ssl = slice(s * NS, (s + 1) * NS)
                zt = rpool.tile([P, NS], BF, tag="z")
                nc.scalar.activation(zt[:], psB[s][:], AF.Sigmoid,
                                     bias=bt[:, GZ * MB + m: GZ * MB + m + 1],
                                     scale=ISC)
                ot = opool.tile([P, NS], F32, tag="o")
                # g-h ; z*(g-h) ; h + z*(g-h)
                nc.vector.tensor_sub(ot[:], gts[s][:], hft[:, ssl])
                nc.vector.tensor_mul(ot[:], zt[:], ot[:])
                nc.vector.tensor_add(ot[:], ot[:], hft[:, ssl])
                nc.sync.dma_start(outT[msl, ssl], ot[:])

    nc.compile()
    return nc


_NC_CACHE = {}


def _get_nc(R):
    if R not in _NC_CACHE:
        _NC_CACHE[R] = build_nc(R)
    return _NC_CACHE[R]


def make_in_maps(update, hidden, wz, uz, bz, wr, ur, br, wh, uh, bh,
                 ncores=NCORES):
    wmap = {}
    for nm, w in (("wz", wz), ("uz", uz), ("wr", wr), ("ur", ur),
                  ("wh", wh), ("uh", uh)):
        wT = np.ascontiguousarray(np.asarray(w, np.float32).T) * WSCALE
        nq = NQ8[nm]
        wmap[nm + "8"] = np.ascontiguousarray(wT[:nq * 2 * P]).astype(f8)
        if nq < NQ:
            wmap[nm + "b"] = np.ascontiguousarray(wT[nq * 2 * P:]).astype(bf16)
    bias = np.empty((P, 3 * MB), np.float32)
    for g, b in enumerate((bz, br, bh)):
        bias[:, g * MB:(g + 1) * MB] = np.asarray(b, np.float32).reshape(MB, P).T
    rows = update.shape[0]
    rc = rows // ncores
    in_maps = []
    for i in range(ncores):
        sl = slice(i * rc, (i + 1) * rc)
        xT = np.ascontiguousarray(np.asarray(update[sl], np.float32).T)
        hT = np.ascontiguousarray(np.asarray(hidden[sl], np.float32).T)
        in_maps.append(dict(
            x8=xT.astype(f8), xb=xT[2 * P:].astype(bf16),
            h8=hT.astype(f8), hb=hT[2 * P:].astype(bf16),
            hf=hT, bias=bias, **wmap))
    return in_maps


def kernel(update, hidden, wz, uz, bz, wr, ur, br, wh, uh, bh):
    global LAST_RESULT
    update = np.asarray(update)
    hidden = np.asarray(hidden)
    R = update.shape[0] // NCORES
    nc = _get_nc(R)
    in_maps = make_in_maps(update, hidden, wz, uz, bz, wr, ur, br, wh, uh, bh)
    res = run_bass_kernel_spmd(nc, in_maps, list(range(NCORES)), trace=TRACE)
    LAST_RESULT = res
    out = np.empty((update.shape[0], H), np.float32)
    for i in range(NCORES):
        out[i * R:(i + 1) * R] = res.results[i]["outT"].T
    return out


# revision 6
# speedup vs baseline: 1.3533x; 1.1163x over previous
"""GRU cell kernel for Trainium2, data-parallel over 8 NeuronCores.

Math (per reference):
    z = sigmoid(x @ wz.T + h @ uz.T + bz)
    r = sigmoid(x @ wr.T + h @ ur.T + br)
    g = tanh(x @ wh.T + (r*h) @ uh.T + bh)
    out = (1-z)*h + z*g = h + z*(g - h)

Everything on-device is computed in TRANSPOSED layout ([feature, row]) so
both matmul operands arrive with the contraction dim on partitions.

Mixed precision: part of the contraction runs as fp8(e4m3) DoubleRow
matmuls (2 MACs/cell/cycle, K=256 per pass), the rest as bf16. Which
k-quarters of each weight matrix are fp8 was chosen by host simulation
to keep max rel err ~0.014 (budget 2e-2):
    wr, ur, uh: all 4 quarters fp8 (r-gate error is attenuated by
        sigmoid slope and the uh moving operand r*h is small in
        magnitude)
    wz, uz, wh: quarter 0 fp8, quarters 1..3 bf16 (z-gate errors are
        amplified by (g - h), tanh has slope 1)
All weights (fp8 and bf16) are pre-scaled by 32 on host (exact in both
formats) so fp8 and bf16 products can share one PSUM accumulation; the
activation undoes it with scale=1/32.

Sharding: rows 16384 -> 8 cores x 2048 rows, weights replicated.
"""

import numpy as np
import ml_dtypes
from contextlib import ExitStack

import concourse.bass as bass
import concourse.bacc as bacc
import concourse.mybir as mybir
import concourse.tile as tile
from concourse.bass_utils import run_bass_kernel_spmd

H = 1024
N_ROWS = 16384
NCORES = 8
P = 128
KB = H // P            # 8 contraction blocks of 128
MB = H // P            # 8 output-feature blocks
NQ = 4                 # k-quarters (256 each)
NS = 512               # rows per matmul moving slice (one PSUM bank)
WSCALE = 32.0          # weight pre-scale (exact power of 2)

# fp8 k-quarters per weight matrix (first nq of 4 quarters are fp8)
NQ8 = {"wz": 1, "uz": 1, "wr": 4, "ur": 4, "wh": 1, "uh": 4}

BF = mybir.dt.bfloat16
F8 = mybir.dt.float8e4
F32 = mybir.dt.float32
AF = mybir.ActivationFunctionType
DR = mybir.MatmulPerfMode.DoubleRow
bf16 = ml_dtypes.bfloat16
f8 = ml_dtypes.float8_e4m3

# Set by test harness to capture a trace; harness-facing default off.
TRACE = False
LAST_RESULT = None


def build_nc(R=N_ROWS // NCORES):
    """Build the per-core Bass program. R rows per core, single chunk."""
    SL = R // NS           # moving slices (4 for R=2048)

    nc = bacc.Bacc(trn_type="TRN2", target_bir_lowering=False,
                   debug=False, enable_asserts=False)

    x8d = nc.dram_tensor("x8", [H, R], F8, kind="ExternalInput").ap()
    xbd = nc.dram_tensor("xb", [6 * P, R], BF, kind="ExternalInput").ap()
    h8d = nc.dram_tensor("h8", [H, R], F8, kind="ExternalInput").ap()
    hbd = nc.dram_tensor("hb", [6 * P, R], BF, kind="ExternalInput").ap()
    hfd = nc.dram_tensor("hf", [H, R], F32, kind="ExternalInput").ap()
    w8d = {}
    wbd = {}
    for nm, nq in NQ8.items():
        w8d[nm] = nc.dram_tensor(nm + "8", [nq * 2 * P, H], F8,
                                 kind="ExternalInput").ap()
        if nq < NQ:
            wbd[nm] = nc.dram_tensor(nm + "b", [(NQ - nq) * 2 * P, H], BF,
                                     kind="ExternalInput").ap()
    bias = nc.dram_tensor("bias", [P, 3 * MB], F32, kind="ExternalInput").ap()
    outT = nc.dram_tensor("outT", [H, R], F32, kind="ExternalOutput").ap()

    with tile.TileContext(nc) as tc, ExitStack() as ctx:
        wpool = ctx.enter_context(tc.tile_pool(name="w", bufs=3))
        dpool = ctx.enter_context(tc.tile_pool(name="d", bufs=1))
        hfpool = ctx.enter_context(tc.tile_pool(name="hf", bufs=2))
        rpool = ctx.enter_context(tc.tile_pool(name="r", bufs=4))
        gpool = ctx.enter_context(tc.tile_pool(name="g", bufs=2 * SL + 2))
        opool = ctx.enter_context(tc.tile_pool(name="o", bufs=4))
        cpool = ctx.enter_context(tc.tile_pool(name="c", bufs=1))
        pspool = ctx.enter_context(tc.tile_pool(name="ps", bufs=8, space="PSUM"))

        # Warm up the ACT table set (sigmoid_and_others covers tanh too) on an
        # instruction with minimal sync waits — walrus can't attach the
        # PSEUDO_LOAD_ACT_FUNC_SET to an activation that already carries two
        # sem waits ("Too many sync wait commands").
        warm = cpool.tile([P, 8], F32, tag="warm")
        nc.gpsimd.memset(warm[:], 0.0)
        nc.scalar.activation(warm[:], warm[:], AF.Sigmoid)

        # Two HW DMA queues: sync (bulk prefetch + output) and scalar
        # (x8/h8 at startup, then the latency-sensitive hf stream). Keeping
        # the hf stream off the bulk queue is essential — behind the 12MB
        # hz prefetch it starves the r-pass rh-multiply for ~30us.
        bt = cpool.tile([P, 3 * MB], F32, tag="bias")
        nc.scalar.dma_start(bt[:], bias[:])
        # bias column layout: [z:0..7 | r:8..15 | h:16..23]
        GZ, GR, GH = 0, 1, 2
        ISC = 1.0 / WSCALE

        # ---- SBUF data tiles + DMA in consumption order ----
        # r-pass m=0 consumes wr/x first, then ur/h; interleave so the PE
        # can start as soon as the first (weight, data) pair lands.
        xt8 = dpool.tile([P, KB, R], F8, tag="x8")
        ht8 = dpool.tile([P, KB, R], F8, tag="h8")
        xtb = dpool.tile([P, 6, R], BF, tag="xb")
        htb = dpool.tile([P, 6, R], BF, tag="hb")
        rht = dpool.tile([P, KB, R], F8, tag="rh")

        w8t = {}
        wbt = {}
        w8t["wr"] = wpool.tile([P, KB, H], F8, tag="w8", name="wr8")
        for k in range(KB):
            ksl = slice(k * P, (k + 1) * P)
            nc.sync.dma_start(w8t["wr"][:, k, :], w8d["wr"][ksl, :])
            nc.scalar.dma_start(xt8[:, k, :], x8d[ksl, :])
        w8t["ur"] = wpool.tile([P, KB, H], F8, tag="w8", name="ur8")
        for k in range(KB):
            ksl = slice(k * P, (k + 1) * P)
            nc.sync.dma_start(w8t["ur"][:, k, :], w8d["ur"][ksl, :])
            nc.scalar.dma_start(ht8[:, k, :], h8d[ksl, :])

        # hz-pass weights + bf16 moving data stream in during the r-pass.
        w8t["uh"] = wpool.tile([P, KB, H], F8, tag="w8", name="uh8")
        for k in range(KB):
            nc.sync.dma_start(w8t["uh"][:, k, :], w8d["uh"][k * P:(k + 1) * P, :])
        for nm in ("wh", "wz", "uz"):
            nq = NQ8[nm]
            w8t[nm] = wpool.tile([P, 2 * nq, H], F8, tag="w8q", name=nm + "8")
            for k in range(2 * nq):
                nc.sync.dma_start(w8t[nm][:, k, :], w8d[nm][k * P:(k + 1) * P, :])
            wbt[nm] = wpool.tile([P, 2 * (NQ - nq), H], BF, tag="wbq", name=nm + "b")
            for k in range(2 * (NQ - nq)):
                nc.sync.dma_start(wbt[nm][:, k, :], wbd[nm][k * P:(k + 1) * P, :])
        for k in range(6):
            nc.sync.dma_start(xtb[:, k, :], xbd[k * P:(k + 1) * P, :])
            nc.sync.dma_start(htb[:, k, :], hbd[k * P:(k + 1) * P, :])

        def mm_fp8(psums, wt, mov, m, nq, start, stop):
            """DoubleRow-accumulate wt.T @ mov for feature block m over
            fp8 k-quarters 0..nq-1."""
            msl = slice(m * P, (m + 1) * P)
            for kq in range(nq):
                for s in range(SL):
                    nc.tensor.matmul(
                        psums[s][:],
                        wt[:, 2 * kq:2 * kq + 2, msl],
                        mov[:, 2 * kq:2 * kq + 2, s * NS:(s + 1) * NS],
                        start=start and kq == 0,
                        stop=stop and kq == nq - 1,
                        perf_mode=DR,
                    )

        def mm_bf16(psums, wt, mov, m, nk, start, stop):
            """bf16-accumulate over nk k-blocks of 128."""
            msl = slice(m * P, (m + 1) * P)
            for k in range(nk):
                for s in range(SL):
                    nc.tensor.matmul(
                        psums[s][:],
                        wt[:, k, msl],
                        mov[:, k, s * NS:(s + 1) * NS],
                        start=start and k == 0,
                        stop=stop and k == nk - 1,
                    )

        # ---- r pass ----
        for m in range(MB):
            hft = hfpool.tile([P, R], F32, tag="hf")
            nc.scalar.dma_start(hft[:], hfd[m * P:(m + 1) * P, :])
            ps = [pspool.tile([P, NS], F32, tag="ps", name="ps") for _ in range(SL)]
            mm_fp8(ps, w8t["wr"], xt8, m, NQ, start=True, stop=False)
            mm_fp8(ps, w8t["ur"], ht8, m, NQ, start=False, stop=True)
            for s in range(SL):
                rt = rpool.tile([P, NS], BF, tag="r")
                nc.scalar.activation(rt[:], ps[s][:], AF.Sigmoid,
                                     bias=bt[:, GR * MB + m: GR * MB + m + 1],
                                     scale=ISC)
                nc.vector.tensor_mul(
                    rht[:, m, s * NS:(s + 1) * NS], rt[:],
                    hft[:, s * NS:(s + 1) * NS])

        # ---- fused h~ / z pass + combine ----
        for m in range(MB):
            msl = slice(m * P, (m + 1) * P)
            hft = hfpool.tile([P, R], F32, tag="hf")
            nc.scalar.dma_start(hft[:], hfd[msl, :])

            psA = [pspool.tile([P, NS], F32, tag="ps", name="psA") for _ in range(SL)]
            mm_fp8(psA, w8t["wh"], xt8, m, NQ8["wh"], start=True, stop=False)
            mm_bf16(psA, wbt["wh"], xtb, m, 2 * (NQ - NQ8["wh"]),
                    start=False, stop=False)
            mm_fp8(psA, w8t["uh"], rht, m, NQ, start=False, stop=True)
            gts = []
            for s in range(SL):
                gt = gpool.tile([P, NS], BF, tag="g")
                nc.scalar.activation(gt[:], psA[s][:], AF.Tanh,
                                     bias=bt[:, GH * MB + m: GH * MB + m + 1],
                                     scale=ISC)
                gts.append(gt)

            psB = [pspool.tile([P, NS], F32, tag="ps", name="psB") for _ in range(SL)]
            mm_fp8(psB, w8t["wz"], xt8, m, NQ8["wz"], start=True, stop=False)
            mm_bf16(psB, wbt["wz"], xtb, m, 2 * (NQ - NQ8["wz"]),
                    start=False, stop=False)
            mm_fp8(psB, w8t["uz"], ht8, m, NQ8["uz"], start=False, stop=False)
            mm_bf16(psB, wbt["uz"], htb, m, 2 * (NQ - NQ8["uz"]),
                    start=False, stop=True)
            for s in range(SL):
                ssl = slice(s * NS, (s + 1) * NS)
                zt = rpool.tile([P, NS], BF, tag="z")
                nc.scalar.activation(zt[:], psB[s][:], AF.Sigmoid,
                                     bias=bt[:, GZ * MB + m: GZ * MB + m + 1],
                                     scale=ISC)
                ot = opool.tile([P, NS], F32, tag="o")
                # g-h ; z*(g-h) ; h + z*(g-h)
                nc.vector.tensor_sub(ot[:], gts[s][:], hft[:, ssl])
                nc.vector.tensor_mul(ot[:], zt[:], ot[:])
                nc.vector.tensor_add(ot[:], ot[:], hft[:, ssl])
                nc.sync.dma_start(outT[msl, ssl], ot[:])

    nc.compile()
    return nc


_NC_CACHE = {}


def _get_nc(R):
    if R not in _NC_CACHE:
        _NC_CACHE[R] = build_nc(R)
    return _NC_CACHE[R]


def make_in_maps(update, hidden, wz, uz, bz, wr, ur, br, wh, uh, bh,
                 ncores=NCORES):
    wmap = {}
    for nm, w in (("wz", wz), ("uz", uz), ("wr", wr), ("ur", ur),
                  ("wh", wh), ("uh", uh)):
        wT = np.ascontiguousarray(np.asarray(w, np.float32).T) * WSCALE
        nq = NQ8[nm]
        wmap[nm + "8"] = np.ascontiguousarray(wT[:nq * 2 * P]).astype(f8)
        if nq < NQ:
            wmap[nm + "b"] = np.ascontiguousarray(wT[nq * 2 * P:]).astype(bf16)
    bias = np.empty((P, 3 * MB), np.float32)
    for g, b in enumerate((bz, br, bh)):
        bias[:, g * MB:(g + 1) * MB] = np.asarray(b, np.float32).reshape(MB, P).T
    rows = update.shape[0]
    rc = rows // ncores
    in_maps = []
    for i in range(ncores):
        sl = slice(i * rc, (i + 1) * rc)
        xT = np.ascontiguousarray(np.asarray(update[sl], np.float32).T)
        hT = np.ascontiguousarray(np.asarray(hidden[sl], np.float32).T)
        in_maps.append(dict(
            x8=xT.astype(f8), xb=xT[2 * P:].astype(bf16),
            h8=hT.astype(f8), hb=hT[2 * P:].astype(bf16),
            hf=hT, bias=bias, **wmap))
    return in_maps


def kernel(update, hidden, wz, uz, bz, wr, ur, br, wh, uh, bh):
    global LAST_RESULT
    update = np.asarray(update)
    hidden = np.asarray(hidden)
    R = update.shape[0] // NCORES
    nc = _get_nc(R)
    in_maps = make_in_maps(update, hidden, wz, uz, bz, wr, ur, br, wh, uh, bh)
    res = run_bass_kernel_spmd(nc, in_maps, list(range(NCORES)), trace=TRACE)
    LAST_RESULT = res
    out = np.empty((update.shape[0], H), np.float32)
    for i in range(NCORES):
        out[i * R:(i + 1) * R] = res.results[i]["outT"].T
    return out
